# revision 2
# baseline (speedup 1.0000x reference)
"""Trainium2 Bass kernel: Backprojection3DConsistencyLoss (8-core SPMD), v5.

v5 = v4's closed-form BCE decomposition, with the cross term |F&L| computed
with ZERO data exchange (v4's AllToAll + transposed send/recv DMAs cost
~350us of descriptor-dominated DMA time on hardware).

Trick: the core-dependent selection that seemed to require a collective is
folded into the per-core one-hot coordinate tables (host inputs). Each core
re-derives, for ALL frontal slice ranks r, a SLOTS-column-wide mini image
  FX_r[c1, xp] (xp < SLOTS),  xp = s  iff  x == hitsL[core + 8*s]
i.e. exactly the columns of frontal slice r at the x-positions of its OWN
lateral slices. That costs two matmuls of free-size SLOTS per (r, batch)
(mm1 batched across all r into one wide matmul per batch; PSUM-side-by-side
mm2 outputs share one sign()), plus one one-hot per r. The cross-term then
pairs FX with the core's own full lateral images:
  X_core = sum_{b, s} sum_{r, c1} FX[b, r, c1, xp=s] * L~_{own s}[c1, zp=r]
(lat images' zp axis is globally frontal-rank-permuted, so zp=r is a literal
index). Everything else (per-slice gt' dots) is unchanged from v4.

All partial sums leave via one [128, 48] tensor; host combines in float64.

If the geometry violates the separability/uniqueness assumptions (checked
exactly on host), a faithful f32 numpy fallback computes the result on host.
"""

import math
import sys

import numpy as np

for _p in ("/opt/trn_rl_repo",):
    if _p not in sys.path:
        sys.path.insert(0, _p)

import concourse.bacc as bacc  # noqa: E402
import concourse.mybir as mybir  # noqa: E402
import concourse.tile as tile  # noqa: E402
from concourse.bass_utils import run_bass_kernel_spmd  # noqa: E402

N_CORES = 8
V = 128          # volume side
S = 512          # samples per ray
POISON = 255.0   # coord value that can never match iota 0..127
F32 = mybir.dt.float32
BF16 = mybir.dt.bfloat16
ALU = mybir.AluOpType

# BCE quadratic: cell loss = Q0 + Q1*s + Q2*s^2 + gt*s, exact for s in {0,1,2}
_B0 = math.log(0.5)
_B1 = -math.log1p(math.e)
_B2 = -2.0 - math.log1p(math.exp(-2.0))
Q0 = _B0
Q1 = (-3.0 * _B0 + 4.0 * _B1 - _B2) / 2.0
Q2 = (_B0 - 2.0 * _B1 + _B2) / 2.0
QC = Q1 + Q2

_PROGRAM_CACHE: dict = {}


class _GeometryFallback(Exception):
    pass


def _build_program(key):
    """key = (hitsF, hitsL): per-view tuples of hit slice indices."""
    if key in _PROGRAM_CACHE:
        return _PROGRAM_CACHE[key]
    hitsF, hitsL = key
    HF, HL = len(hitsF), len(hitsL)
    SLOTS = -(-max(HF, HL) // N_CORES)
    WR = N_CORES * SLOTS         # padded frontal rank width (<= 128)
    NCOL = 48                    # acc columns (42 used)
    RCH = max(1, min(WR, 448 // SLOTS))   # r-chunk so PSUM tile <= 448 f32

    nc = bacc.Bacc("TRN2", target_bir_lowering=False, debug=False,
                   num_devices=N_CORES)
    masks = nc.declare_dram_parameter("masks", [128, 4, 128], BF16,
                                      isOutput=False)
    rc = nc.declare_dram_parameter("rc", [128, 2, SLOTS, 2], F32,
                                   isOutput=False)
    # rx[:, 0, r] = own-lateral-slot selector of frontal row coord (x -> xp)
    # rx[:, 1, r] = frontal col coord (y = c1), raw
    rx = nc.declare_dram_parameter("rx", [128, 2, WR], F32, isOutput=False)
    gtp = nc.declare_dram_parameter("gtp", [128, 2, SLOTS, 128], F32,
                                    isOutput=False)
    out_vec = nc.declare_dram_parameter("out_vec", [128, NCOL], F32,
                                        isOutput=True)

    with tile.TileContext(nc) as tc:
        with (
            tc.tile_pool(name="const", bufs=1) as constp,
            tc.tile_pool(name="oh", bufs=4) as ohp,
            tc.tile_pool(name="cp", bufs=4) as cpp,
            tc.tile_pool(name="psum", bufs=2, space="PSUM") as psump,
            tc.tile_pool(name="psx", bufs=1, space="PSUM") as psxp,
            tc.tile_pool(name="limg", bufs=1) as limgp,
            tc.tile_pool(name="scr", bufs=4) as scrp,
            tc.tile_pool(name="io", bufs=1) as iop,
        ):
            iota_i = constp.tile([128, 128], mybir.dt.int32)
            nc.gpsimd.iota(iota_i[:], pattern=[[1, 128]], base=0,
                           channel_multiplier=0)
            iota_b = constp.tile([128, 128], BF16)
            nc.vector.tensor_copy(iota_b[:], iota_i[:])

            masks_sb = constp.tile([128, 4, 128], BF16)
            nc.sync.dma_start(masks_sb[:], masks.ap())
            rc_sb = constp.tile([128, 2, SLOTS, 2], F32)
            nc.sync.dma_start(rc_sb[:], rc.ap())
            rx_sb = constp.tile([128, 2, WR], F32)
            nc.sync.dma_start(rx_sb[:], rx.ap())
            gtp_sb = constp.tile([128, 2, SLOTS, 128], F32)
            nc.sync.dma_start(gtp_sb[:], gtp.ap())

            acc_t = iop.tile([128, NCOL], F32, name="acc", tag="acc")
            nc.vector.memset(acc_t[:], 0.0)

            limg = {}
            # ---- main slices: own 7 per view, full images + gt' dots
            for view in (1, 0):
                for sl in range(SLOTS):
                    ohr = ohp.tile([128, 128], BF16, tag="ohr")
                    nc.vector.tensor_scalar(
                        ohr[:], iota_b[:], rc_sb[:, view, sl, 0:1], None,
                        ALU.is_equal)
                    ohc = ohp.tile([128, 128], BF16, tag="ohc")
                    nc.vector.tensor_scalar(
                        ohc[:], iota_b[:], rc_sb[:, view, sl, 1:2], None,
                        ALU.is_equal)
                    for b in range(2):
                        v = 2 * b + view
                        ps1 = psump.tile([128, 128], F32)
                        # out1[j, n] = sum_i A[i, j] * ohr[i, n]
                        nc.tensor.matmul(ps1[:], lhsT=masks_sb[:, v, :],
                                         rhs=ohr[:], start=True, stop=True)
                        t1 = cpp.tile([128, 128], BF16, tag="t1")
                        nc.scalar.copy(t1[:], ps1[:])
                        ps2 = psump.tile([128, 128], F32)
                        if view == 0:
                            # F~[c1, x] = sum_j ohc[j, c1] * out1[j, x]
                            nc.tensor.matmul(ps2[:], lhsT=ohc[:], rhs=t1[:],
                                             start=True, stop=True)
                        else:
                            # L~[c1, zp] = sum_j out1[j, c1] * ohc[j, zp]
                            nc.tensor.matmul(ps2[:], lhsT=t1[:], rhs=ohc[:],
                                             start=True, stop=True)
                        if view == 1:
                            img = limgp.tile([128, 128], BF16,
                                             name=f"limg{b}_{sl}",
                                             tag=f"li{b}_{sl}")
                            limg[(b, sl)] = img
                        else:
                            img = scrp.tile([128, 128], BF16, tag="fimg")
                        nc.scalar.sign(img[:], ps2[:])
                        col = (view * 2 * SLOTS) + sl * 2 + b
                        dot_out = scrp.tile([128, 128], F32, tag="dot")
                        nc.vector.scalar_tensor_tensor(
                            out=dot_out[:], in0=img[:], scalar=1.0,
                            in1=gtp_sb[:, view, sl, :],
                            op0=ALU.mult, op1=ALU.mult,
                            accum_out=acc_t[:, col:col + 1])

            # ---- FX phase: SLOTS-wide frontal mini-images for ALL ranks
            # wide one-hot: ohxw[i, r, xp] = [xp == rx[i, 0, r]]
            iota7 = constp.tile([128, SLOTS], BF16, name="iota7", tag="i7")
            nc.vector.tensor_copy(iota7[:], iota_i[:, 0:SLOTS])
            ohxw = iop.tile([128, WR, SLOTS], BF16, name="ohxw", tag="ohxw")
            nc.vector.tensor_tensor(
                ohxw[:],
                iota7[:].unsqueeze(1).broadcast_to([128, WR, SLOTS]),
                rx_sb[:, 0, :].unsqueeze(2).broadcast_to([128, WR, SLOTS]),
                ALU.is_equal)
            FX = iop.tile([128, 2, WR, SLOTS], BF16, name="FX", tag="FX")
            for r0 in range(0, WR, RCH):
                r1 = min(r0 + RCH, WR)
                t1x = {}
                ps2x = {}
                for b in range(2):
                    ps1x = psxp.tile([128, (r1 - r0) * SLOTS], F32,
                                     name=f"ps1x{b}", tag=f"ps1x{b}")
                    nc.tensor.matmul(
                        ps1x[:], lhsT=masks_sb[:, 2 * b, :],
                        rhs=ohxw[:, r0:r1, :].rearrange("p r x -> p (r x)"),
                        start=True, stop=True)
                    t1x[b] = cpp.tile([128, (r1 - r0) * SLOTS], BF16,
                                      name=f"t1x{b}", tag=f"t1x{b}")
                    nc.scalar.copy(t1x[b][:], ps1x[:])
                    ps2x[b] = psxp.tile([128, (r1 - r0) * SLOTS], F32,
                                        name=f"ps2x{b}", tag=f"ps2x{b}")
                for r in range(r0, r1):
                    ohcf = ohp.tile([128, 128], BF16, tag="ohcf")
                    nc.vector.tensor_scalar(
                        ohcf[:], iota_b[:], rx_sb[:, 1, r:r + 1], None,
                        ALU.is_equal)
                    for b in range(2):
                        o = (r - r0) * SLOTS
                        nc.tensor.matmul(
                            ps2x[b][:, o:o + SLOTS], lhsT=ohcf[:],
                            rhs=t1x[b][:, o:o + SLOTS],
                            start=True, stop=True)
                for b in range(2):
                    nc.scalar.sign(
                        FX[:, b, r0:r1, :].rearrange("p r x -> p (r x)"),
                        ps2x[b][:])

            # ---- cross-term dots: own lateral slice s pairs with FX[.., s]
            for b in range(2):
                for s in range(SLOTS):
                    in0 = FX[:, b, :, s]
                    in1 = limg[(b, s)][:, 0:WR]
                    xout = scrp.tile([128, WR], BF16, tag="xout")
                    col = 4 * SLOTS + s * 2 + b
                    nc.vector.scalar_tensor_tensor(
                        out=xout[:], in0=in0, scalar=1.0, in1=in1,
                        op0=ALU.mult, op1=ALU.mult,
                        accum_out=acc_t[:, col:col + 1])

            nc.sync.dma_start(out_vec.ap(), acc_t[:])

    nc.compile()
    _PROGRAM_CACHE[key] = nc
    return nc


def _trace_view(src, tgt, A_inv, t_inv):
    """f32 mirror of the reference ray-march for ALL detector pixels."""
    f32 = np.float32
    det = tgt.reshape(-1, 3).astype(f32)
    rd = (det - src[None, :]).astype(f32)
    rl = np.sqrt((rd * rd).sum(1, dtype=f32)).astype(f32)[:, None]
    rdn = (rd / (rl + f32(1e-8))).astype(f32)
    tv = np.linspace(0.0, 1.0, S).astype(f32)
    ts = (tv[None, :, None] * (rl[:, None, :] * f32(2.5))).astype(f32)
    world = (src[None, None, :] + rdn[:, None, :] * ts).astype(f32)
    vox_f = (world @ A_inv.T + t_inv).astype(f32)
    vox = np.rint(vox_f).astype(np.int32)
    ok = ((vox[..., 0] >= 0) & (vox[..., 0] < V)
          & (vox[..., 1] >= 0) & (vox[..., 1] < V)
          & (vox[..., 2] >= 0) & (vox[..., 2] < V))
    return vox, ok


def _view_tables(vox, ok, ax, m_ax, n_ax):
    """Separable per-slice coord tables for one view (see v3 docstring)."""
    P = vox.shape[0]
    k_arr = vox[..., ax]
    rr, ss = np.nonzero(ok)
    kk = k_arr[rr, ss]
    counts = np.zeros((P, V), dtype=np.int32)
    np.add.at(counts, (rr, kk), 1)
    if counts.max(initial=0) > 1:
        raise _GeometryFallback("duplicate samples per (ray, slice)")
    mk = np.full((P, V), POISON, dtype=np.float32)
    nk = np.full((P, V), POISON, dtype=np.float32)
    mk[rr, kk] = vox[..., m_ax][rr, ss]
    nk[rr, kk] = vox[..., n_ax][rr, ss]
    hits = tuple(int(k) for k in np.flatnonzero(counts.any(axis=0)))

    mk3 = mk.reshape(128, 128, V)     # [i, j, k]; m varies with j
    nk3 = nk.reshape(128, 128, V)     # n varies with i
    coltab_full = mk3.min(axis=0)     # [j, k]
    rowtab_full = nk3.min(axis=1)     # [i, k]
    pred_valid = ((coltab_full[None, :, :] != POISON)
                  & (rowtab_full[:, None, :] != POISON))
    pm = np.where(pred_valid, coltab_full[None, :, :], POISON)
    pn = np.where(pred_valid, rowtab_full[:, None, :], POISON)
    ndev = int((pm != mk3).sum() + (pn != nk3).sum())
    ks = np.asarray(hits, dtype=np.int64)
    return hits, rowtab_full[:, ks], coltab_full[:, ks], ndev


def _perm_table(tab, perm):
    """Apply permutation to non-POISON entries of a coord table column."""
    f32 = np.float32
    return np.where(tab != POISON,
                    perm[np.clip(tab.astype(np.int64), 0, V - 1)].astype(f32),
                    f32(POISON))


def _host_prep(inputs):
    f32 = np.float32
    pf = np.asarray(inputs["pred_frontal"], dtype=f32)
    pl = np.asarray(inputs["pred_lateral"], dtype=f32)
    srcF = np.asarray(inputs["source_F"], dtype=f32)[0]
    tgtF = np.asarray(inputs["target_F"], dtype=f32)[0]
    srcL = np.asarray(inputs["source_L"], dtype=f32)[0]
    tgtL = np.asarray(inputs["target_L"], dtype=f32)[0]
    A_inv = np.asarray(inputs["A_inv"], dtype=f32)
    t_inv = np.asarray(inputs["t_inv"], dtype=f32)
    gt = np.asarray(inputs["vol_gt_3d"], dtype=f32)
    B = pf.shape[0]
    if B != 2 or gt.shape != (V, V, V) or pf.shape[2] != V:
        raise _GeometryFallback(f"unexpected shapes B={B}")

    voxF, okF = _trace_view(srcF, tgtF, A_inv, t_inv)
    voxL, okL = _trace_view(srcL, tgtL, A_inv, t_inv)
    stepsF = np.abs(np.diff(voxF.astype(np.int64), axis=1)).mean((0, 1))
    if int(np.argmax(stepsF)) != 2:
        raise _GeometryFallback("frontal dominant axis not z")
    stepsL = np.abs(np.diff(voxL.astype(np.int64), axis=1)).mean((0, 1))
    if int(np.argmax(stepsL)) != 0:
        raise _GeometryFallback("lateral dominant axis not x")

    # frontal: slice axis c2; row table -> x (c0), col table -> y (c1)
    hitsF, rowF, colF, devF = _view_tables(voxF, okF, 2, 1, 0)
    # lateral: slice axis c0; row table -> y (c1), col table -> z (c2)
    hitsL, rowL, colL, devL = _view_tables(voxL, okL, 0, 2, 1)
    if devF + devL > 2000:
        raise _GeometryFallback(f"separability violated ({devF}+{devL})")

    HF, HL = len(hitsF), len(hitsL)
    if HF == 0 or HL == 0 or HF > V or HL > V:
        raise _GeometryFallback("degenerate hit sets")
    SLOTS = -(-max(HF, HL) // N_CORES)
    WR = N_CORES * SLOTS

    # global z-permutation: frontal hit slices rank-first
    permzF = np.full(V, -1, dtype=np.int64)
    for r, k in enumerate(hitsF):
        permzF[k] = r
    nxt = HF
    for k in range(V):
        if permzF[k] < 0:
            permzF[k] = nxt
            nxt += 1
    zinv = np.argsort(permzF)          # zp -> z

    # per-core inputs
    in_maps = []
    for c in range(N_CORES):
        rc_arr = np.full((128, 2, SLOTS, 2), POISON, dtype=f32)
        gtp_arr = np.zeros((128, 2, SLOTS, 128), dtype=f32)
        rx_arr = np.full((128, 2, WR), POISON, dtype=f32)
        # own-lateral-slot selector: x -> xp (= slot if x is mine else SLOTS)
        permxLc = np.full(V, SLOTS, dtype=np.int64)
        for s in range(SLOTS):
            xr = c + N_CORES * s
            if xr < HL:
                permxLc[hitsL[xr]] = s
        for s in range(SLOTS):
            r = c + N_CORES * s
            if r < HF:                  # own frontal slice rank r
                rc_arr[:, 0, s, 0] = rowF[:, r]            # x raw
                rc_arr[:, 0, s, 1] = colF[:, r]            # y = c1 raw
                z = hitsF[r]
                gtp_arr[:, 0, s, :] = gt[:, :, z].T + f32(QC)
            if r < HL:                  # own lateral slice rank r
                rc_arr[:, 1, s, 0] = rowL[:, r]            # y = c1 raw
                rc_arr[:, 1, s, 1] = _perm_table(colL[:, r],
                                                 permzF)   # z -> zp
                x = hitsL[r]
                gtp_arr[:, 1, s, :] = gt[x][:, zinv] + f32(QC)
        for r in range(min(HF, WR)):    # FX tables for ALL frontal ranks
            rx_arr[:, 0, r] = _perm_table(rowF[:, r], permxLc)
            rx_arr[:, 1, r] = colF[:, r]
        mk_arr = np.zeros((128, 4, 128), dtype=f32)
        for b in range(2):
            mk_arr[:, 2 * b + 0, :] = (pf[b, 0] > 0.5)
            mk_arr[:, 2 * b + 1, :] = (pl[b, 0] > 0.5)
        in_maps.append({
            "masks": mk_arr.astype(mybir.dt.np(BF16)),
            "rc": rc_arr,
            "rx": rx_arr,
            "gtp": gtp_arr,
        })
    return in_maps, (tuple(hitsF), tuple(hitsL))


def _combine(out_vecs, slots):
    """Host combine of per-core partial sums (float64)."""
    ov = np.asarray(out_vecs, dtype=np.float64)   # [cores, 128, NCOL]
    S1 = ov[:, :, 0:4 * slots].sum()              # gt'-dots (both views)
    S2 = ov[:, :, 4 * slots:6 * slots].sum()      # |F&L| partials
    N = float(V) ** 3
    total = 2.0 * N * Q0 + S1 + 2.0 * Q2 * S2
    return np.float32(-total / (2.0 * N))


def _reference_fallback(inputs):
    """Faithful f32 numpy replica of the jax reference (safety net)."""
    f32 = np.float32
    pf = np.asarray(inputs["pred_frontal"], dtype=f32)
    pl = np.asarray(inputs["pred_lateral"], dtype=f32)
    srcF = np.asarray(inputs["source_F"], dtype=f32)[0]
    tgtF = np.asarray(inputs["target_F"], dtype=f32)[0]
    srcL = np.asarray(inputs["source_L"], dtype=f32)[0]
    tgtL = np.asarray(inputs["target_L"], dtype=f32)[0]
    A_inv = np.asarray(inputs["A_inv"], dtype=f32)
    t_inv = np.asarray(inputs["t_inv"], dtype=f32)
    gt = np.asarray(inputs["vol_gt_3d"], dtype=f32)

    def backproject(mask2d, src, tgt):
        vox, ok = _trace_view(src, tgt, A_inv, t_inv)
        active = (mask2d > 0.5).reshape(-1)
        okm = ok & active[:, None]
        vi = np.clip(vox, 0, V - 1)
        vol = np.zeros((V, V, V), dtype=f32)
        flat = (vi[..., 0] * V + vi[..., 1]) * V + vi[..., 2]
        vol.reshape(-1)[flat[okm]] = 1.0
        return vol

    total = 0.0
    B = pf.shape[0]
    for b in range(B):
        vF = backproject(pf[b, 0], srcF, tgtF)
        vL = backproject(pl[b, 0], srcL, tgtL)
        sv = (vF + vL).astype(np.float64)
        p = 1.0 / (1.0 + np.exp(-sv))
        total += -(gt * np.log(p) + (1.0 - gt) * np.log1p(-p)).mean()
    return np.float32(total / B)


def kernel(**inputs) -> np.ndarray:
    try:
        in_maps, key = _host_prep(inputs)
    except _GeometryFallback:
        return _reference_fallback(inputs)
    nc = _build_program(key)
    res = run_bass_kernel_spmd(nc, in_maps, list(range(N_CORES)))
    slots = -(-max(len(key[0]), len(key[1])) // N_CORES)
    return _combine([r["out_vec"] for r in res.results], slots)


# revision 3
# speedup vs baseline: 1.2494x; 1.2494x over previous
"""Trainium2 Bass kernel: Backprojection3DConsistencyLoss (8-core SPMD), v5.

v7 = v5's zero-collective decomposition with the PE critical path cut:
mm1 batched across slots into PSUM-bank-wide matmuls, frontal mm2 batched
across batch (shared stationary), FX mm2 batched across batch, sign()
grouped per PSUM bank. Only contiguous/2-3D APs (HW verifier safe); the
cross term |F&L| is computed with ZERO data exchange (v4's AllToAll + transposed send/recv DMAs cost
~350us of descriptor-dominated DMA time on hardware).

Trick: the core-dependent selection that seemed to require a collective is
folded into the per-core one-hot coordinate tables (host inputs). Each core
re-derives, for ALL frontal slice ranks r, a SLOTS-column-wide mini image
  FX_r[c1, xp] (xp < SLOTS),  xp = s  iff  x == hitsL[core + 8*s]
i.e. exactly the columns of frontal slice r at the x-positions of its OWN
lateral slices. That costs two matmuls of free-size SLOTS per (r, batch)
(mm1 batched across all r into one wide matmul per batch; PSUM-side-by-side
mm2 outputs share one sign()), plus one one-hot per r. The cross-term then
pairs FX with the core's own full lateral images:
  X_core = sum_{b, s} sum_{r, c1} FX[b, r, c1, xp=s] * L~_{own s}[c1, zp=r]
(lat images' zp axis is globally frontal-rank-permuted, so zp=r is a literal
index). Everything else (per-slice gt' dots) is unchanged from v4.

All partial sums leave via one [128, 48] tensor; host combines in float64.

If the geometry violates the separability/uniqueness assumptions (checked
exactly on host), a faithful f32 numpy fallback computes the result on host.
"""

import math
import sys

import numpy as np

for _p in ("/opt/trn_rl_repo",):
    if _p not in sys.path:
        sys.path.insert(0, _p)

import concourse.bacc as bacc  # noqa: E402
import concourse.mybir as mybir  # noqa: E402
import concourse.tile as tile  # noqa: E402
from concourse.bass_utils import run_bass_kernel_spmd  # noqa: E402

N_CORES = 8
V = 128          # volume side
S = 512          # samples per ray
POISON = 255.0   # coord value that can never match iota 0..127
F32 = mybir.dt.float32
BF16 = mybir.dt.bfloat16
ALU = mybir.AluOpType

# BCE quadratic: cell loss = Q0 + Q1*s + Q2*s^2 + gt*s, exact for s in {0,1,2}
_B0 = math.log(0.5)
_B1 = -math.log1p(math.e)
_B2 = -2.0 - math.log1p(math.exp(-2.0))
Q0 = _B0
Q1 = (-3.0 * _B0 + 4.0 * _B1 - _B2) / 2.0
Q2 = (_B0 - 2.0 * _B1 + _B2) / 2.0
QC = Q1 + Q2

_PROGRAM_CACHE: dict = {}


class _GeometryFallback(Exception):
    pass


def _build_program(key):
    """key = (hitsF, hitsL): per-view tuples of hit slice indices."""
    if key in _PROGRAM_CACHE:
        return _PROGRAM_CACHE[key]
    hitsF, hitsL = key
    HF, HL = len(hitsF), len(hitsL)
    SLOTS = -(-max(HF, HL) // N_CORES)
    WR = N_CORES * SLOTS         # padded frontal rank width (<= 128)
    NCOL = 48                    # acc columns (42 used)
    RCH = max(1, min(WR, 448 // SLOTS))   # r-chunk so PSUM tile <= 448 f32

    nc = bacc.Bacc("TRN2", target_bir_lowering=False, debug=False,
                   num_devices=N_CORES)
    masks = nc.declare_dram_parameter("masks", [128, 4, 128], BF16,
                                      isOutput=False)
    rc = nc.declare_dram_parameter("rc", [128, 2, SLOTS, 2], F32,
                                   isOutput=False)
    # rx[:, 0, r] = own-lateral-slot selector of frontal row coord (x -> xp)
    # rx[:, 1, r] = frontal col coord (y = c1), raw
    rx = nc.declare_dram_parameter("rx", [128, 2, WR], F32, isOutput=False)
    gtp = nc.declare_dram_parameter("gtp", [128, 2, SLOTS, 128], F32,
                                    isOutput=False)
    out_vec = nc.declare_dram_parameter("out_vec", [128, NCOL], F32,
                                        isOutput=True)

    with tile.TileContext(nc) as tc:
        with (
            tc.tile_pool(name="const", bufs=1) as constp,
            tc.tile_pool(name="oh", bufs=1) as ohp,
            tc.tile_pool(name="cp", bufs=1) as cpp,
            tc.tile_pool(name="ps1", bufs=2, space="PSUM") as ps1p,
            tc.tile_pool(name="ps2", bufs=2, space="PSUM") as ps2p,
            tc.tile_pool(name="psx", bufs=1, space="PSUM") as psxp,
            tc.tile_pool(name="scr", bufs=4) as scrp,
            tc.tile_pool(name="io", bufs=1) as iop,
        ):
            BANK = 512
            SL_CH = max(1, BANK // 128)
            iota_i = constp.tile([128, 128], mybir.dt.int32)
            nc.gpsimd.iota(iota_i[:], pattern=[[1, 128]], base=0,
                           channel_multiplier=0)
            iota_b = constp.tile([128, 128], BF16)
            nc.vector.tensor_copy(iota_b[:], iota_i[:])

            masks_sb = constp.tile([128, 4, 128], BF16)
            nc.sync.dma_start(masks_sb[:], masks.ap())
            rc_sb = constp.tile([128, 2, SLOTS, 2], F32)
            nc.sync.dma_start(rc_sb[:], rc.ap())
            rx_sb = constp.tile([128, 2, WR], F32)
            nc.sync.dma_start(rx_sb[:], rx.ap())
            gtp_sb = constp.tile([128, 2, SLOTS, 128], F32)
            nc.sync.dma_start(gtp_sb[:], gtp.ap())

            acc_t = iop.tile([128, NCOL], F32, name="acc", tag="acc")
            nc.vector.memset(acc_t[:], 0.0)

            # per-slot one-hots written into wide tiles (contiguous APs only)
            ohrw = ohp.tile([128, 2, SLOTS, 128], BF16, name="ohrw",
                            tag="ohrw")
            ohcw = ohp.tile([128, 2, SLOTS, 128], BF16, name="ohcw",
                            tag="ohcw")
            for view in range(2):
                for sl in range(SLOTS):
                    nc.vector.tensor_scalar(
                        ohrw[:, view, sl, :], iota_b[:],
                        rc_sb[:, view, sl, 0:1], None, ALU.is_equal)
                    nc.vector.tensor_scalar(
                        ohcw[:, view, sl, :], iota_b[:],
                        rc_sb[:, view, sl, 1:2], None, ALU.is_equal)

            # mm1 batched across slots; t1w[c1, view, sl, b, n]
            t1w = cpp.tile([128, 2, SLOTS, 2, 128], BF16, name="t1w",
                           tag="t1w")
            for view in range(2):
                for b in range(2):
                    v = 2 * b + view
                    for s0 in range(0, SLOTS, SL_CH):
                        s1 = min(s0 + SL_CH, SLOTS)
                        ps1 = ps1p.tile([128, (s1 - s0) * 128], F32,
                                        name="ps1", tag="ps1")
                        nc.tensor.matmul(
                            ps1[:], lhsT=masks_sb[:, v, :],
                            rhs=ohrw[:, view, s0:s1, :].rearrange(
                                "p s n -> p (s n)"),
                            start=True, stop=True)
                        nc.scalar.copy(t1w[:, view, s0:s1, b, :], ps1[:])

            # mm2: frontal batched across b (shared stationary one-hot),
            # lateral per (sl, b); sign() grouped per PSUM bank.
            # img_w[c1, view, sl, b, n]
            img_w = iop.tile([128, 2, SLOTS, 2, 128], BF16, name="img_w",
                             tag="img_w")
            img_flat = img_w[:].rearrange("p v s b n -> p (v s b n)")
            GROUP = BANK // 128          # 128-col units per PSUM bank
            pend = []                    # (psum_tile, filled_units, flat0)

            def flush():
                for psg, used, flat0 in pend:
                    nc.scalar.sign(img_flat[:, flat0:flat0 + used * 128],
                                   psg[:, 0:used * 128])
                pend.clear()

            unit = 0                     # running 128-col unit index
            psg = None
            for view in range(2):
                for sl in range(SLOTS):
                    take = 2 if view == 0 else 1
                    for b in range(0, 2, take):
                        if psg is None or unit % GROUP == 0:
                            if psg is not None:
                                pend.append((psg, GROUP, flat0))
                            if len(pend) >= 2:
                                flush()
                            psg = ps2p.tile([128, BANK], F32, name="psg",
                                            tag="psg")
                            flat0 = unit * 128
                            filled = 0
                        o = (unit % GROUP) * 128
                        if view == 0:
                            # F~[c1, x] both batches at once
                            nc.tensor.matmul(
                                psg[:, o:o + 256], lhsT=ohcw[:, 0, sl, :],
                                rhs=t1w[:, 0, sl, :, :].rearrange(
                                    "p b n -> p (b n)"),
                                start=True, stop=True)
                        else:
                            nc.tensor.matmul(
                                psg[:, o:o + 128],
                                lhsT=t1w[:, 1, sl, b, :],
                                rhs=ohcw[:, 1, sl, :],
                                start=True, stop=True)
                        unit += take
                        filled = unit * 128 - flat0
            if psg is not None:
                pend.append((psg, filled // 128, flat0))
            flush()

            # per-slice gt' dots (v5 style; contiguous 2D APs)
            for view in range(2):
                for sl in range(SLOTS):
                    for b in range(2):
                        col = (view * 2 * SLOTS) + sl * 2 + b
                        dot_out = scrp.tile([128, 128], F32, tag="dot")
                        nc.vector.scalar_tensor_tensor(
                            out=dot_out[:], in0=img_w[:, view, sl, b, :],
                            scalar=1.0, in1=gtp_sb[:, view, sl, :],
                            op0=ALU.mult, op1=ALU.mult,
                            accum_out=acc_t[:, col:col + 1])

            # ---- FX phase: SLOTS-wide frontal mini-images for ALL ranks
            iota7 = constp.tile([128, SLOTS], BF16, name="iota7", tag="i7")
            nc.vector.tensor_copy(iota7[:], iota_i[:, 0:SLOTS])
            ohxw = iop.tile([128, WR, SLOTS], BF16, name="ohxw", tag="ohxw")
            nc.vector.tensor_tensor(
                ohxw[:],
                iota7[:].unsqueeze(1).broadcast_to([128, WR, SLOTS]),
                rx_sb[:, 0, :].unsqueeze(2).broadcast_to([128, WR, SLOTS]),
                ALU.is_equal)
            # t1x[c1, r, b, xp]
            t1x = cpp.tile([128, WR, 2, SLOTS], BF16, name="t1x", tag="t1x")
            RCH1 = max(1, BANK // SLOTS)
            for b in range(2):
                for r0 in range(0, WR, RCH1):
                    r1 = min(r0 + RCH1, WR)
                    ps1x = psxp.tile([128, (r1 - r0) * SLOTS], F32,
                                     name=f"ps1x{b}", tag=f"ps1x{b}")
                    nc.tensor.matmul(
                        ps1x[:], lhsT=masks_sb[:, 2 * b, :],
                        rhs=ohxw[:, r0:r1, :].rearrange("p r x -> p (r x)"),
                        start=True, stop=True)
                    nc.scalar.copy(t1x[:, r0:r1, b, :], ps1x[:])
            # FX[c1, r, b, xp]; mm2 batched across b, signs per PSUM chunk
            FX = iop.tile([128, WR, 2, SLOTS], BF16, name="FX", tag="FX")
            RCH2 = max(1, BANK // (2 * SLOTS))
            for r0 in range(0, WR, RCH2):
                r1 = min(r0 + RCH2, WR)
                ps2x = psxp.tile([128, (r1 - r0) * 2 * SLOTS], F32,
                                 name="ps2x", tag="ps2x")
                for r in range(r0, r1):
                    ohcf = scrp.tile([128, 128], BF16, tag="ohcf")
                    nc.vector.tensor_scalar(
                        ohcf[:], iota_b[:], rx_sb[:, 1, r:r + 1], None,
                        ALU.is_equal)
                    o = (r - r0) * 2 * SLOTS
                    nc.tensor.matmul(
                        ps2x[:, o:o + 2 * SLOTS], lhsT=ohcf[:],
                        rhs=t1x[:, r, :, :].rearrange("p b x -> p (b x)"),
                        start=True, stop=True)
                nc.scalar.sign(
                    FX[:, r0:r1, :, :].rearrange("p r b x -> p (r b x)"),
                    ps2x[:])

            # ---- cross-term dots (2D APs)
            for b in range(2):
                for s in range(SLOTS):
                    in0 = FX[:, :, b, s]
                    in1 = img_w[:, 1, s, b, 0:WR]
                    xout = scrp.tile([128, WR], BF16, tag="xout")
                    col = 4 * SLOTS + s * 2 + b
                    nc.vector.scalar_tensor_tensor(
                        out=xout[:], in0=in0, scalar=1.0, in1=in1,
                        op0=ALU.mult, op1=ALU.mult,
                        accum_out=acc_t[:, col:col + 1])

            nc.sync.dma_start(out_vec.ap(), acc_t[:])

    nc.compile()
    _PROGRAM_CACHE[key] = nc
    return nc


def _trace_view(src, tgt, A_inv, t_inv):
    """f32 mirror of the reference ray-march for ALL detector pixels."""
    f32 = np.float32
    det = tgt.reshape(-1, 3).astype(f32)
    rd = (det - src[None, :]).astype(f32)
    rl = np.sqrt((rd * rd).sum(1, dtype=f32)).astype(f32)[:, None]
    rdn = (rd / (rl + f32(1e-8))).astype(f32)
    tv = np.linspace(0.0, 1.0, S).astype(f32)
    ts = (tv[None, :, None] * (rl[:, None, :] * f32(2.5))).astype(f32)
    world = (src[None, None, :] + rdn[:, None, :] * ts).astype(f32)
    vox_f = (world @ A_inv.T + t_inv).astype(f32)
    vox = np.rint(vox_f).astype(np.int32)
    ok = ((vox[..., 0] >= 0) & (vox[..., 0] < V)
          & (vox[..., 1] >= 0) & (vox[..., 1] < V)
          & (vox[..., 2] >= 0) & (vox[..., 2] < V))
    return vox, ok


def _view_tables(vox, ok, ax, m_ax, n_ax):
    """Separable per-slice coord tables for one view (see v3 docstring)."""
    P = vox.shape[0]
    k_arr = vox[..., ax]
    rr, ss = np.nonzero(ok)
    kk = k_arr[rr, ss]
    counts = np.zeros((P, V), dtype=np.int32)
    np.add.at(counts, (rr, kk), 1)
    if counts.max(initial=0) > 1:
        raise _GeometryFallback("duplicate samples per (ray, slice)")
    mk = np.full((P, V), POISON, dtype=np.float32)
    nk = np.full((P, V), POISON, dtype=np.float32)
    mk[rr, kk] = vox[..., m_ax][rr, ss]
    nk[rr, kk] = vox[..., n_ax][rr, ss]
    hits = tuple(int(k) for k in np.flatnonzero(counts.any(axis=0)))

    mk3 = mk.reshape(128, 128, V)     # [i, j, k]; m varies with j
    nk3 = nk.reshape(128, 128, V)     # n varies with i
    coltab_full = mk3.min(axis=0)     # [j, k]
    rowtab_full = nk3.min(axis=1)     # [i, k]
    pred_valid = ((coltab_full[None, :, :] != POISON)
                  & (rowtab_full[:, None, :] != POISON))
    pm = np.where(pred_valid, coltab_full[None, :, :], POISON)
    pn = np.where(pred_valid, rowtab_full[:, None, :], POISON)
    ndev = int((pm != mk3).sum() + (pn != nk3).sum())
    ks = np.asarray(hits, dtype=np.int64)
    return hits, rowtab_full[:, ks], coltab_full[:, ks], ndev


def _perm_table(tab, perm):
    """Apply permutation to non-POISON entries of a coord table column."""
    f32 = np.float32
    return np.where(tab != POISON,
                    perm[np.clip(tab.astype(np.int64), 0, V - 1)].astype(f32),
                    f32(POISON))


def _host_prep(inputs):
    f32 = np.float32
    pf = np.asarray(inputs["pred_frontal"], dtype=f32)
    pl = np.asarray(inputs["pred_lateral"], dtype=f32)
    srcF = np.asarray(inputs["source_F"], dtype=f32)[0]
    tgtF = np.asarray(inputs["target_F"], dtype=f32)[0]
    srcL = np.asarray(inputs["source_L"], dtype=f32)[0]
    tgtL = np.asarray(inputs["target_L"], dtype=f32)[0]
    A_inv = np.asarray(inputs["A_inv"], dtype=f32)
    t_inv = np.asarray(inputs["t_inv"], dtype=f32)
    gt = np.asarray(inputs["vol_gt_3d"], dtype=f32)
    B = pf.shape[0]
    if B != 2 or gt.shape != (V, V, V) or pf.shape[2] != V:
        raise _GeometryFallback(f"unexpected shapes B={B}")

    voxF, okF = _trace_view(srcF, tgtF, A_inv, t_inv)
    voxL, okL = _trace_view(srcL, tgtL, A_inv, t_inv)
    stepsF = np.abs(np.diff(voxF.astype(np.int64), axis=1)).mean((0, 1))
    if int(np.argmax(stepsF)) != 2:
        raise _GeometryFallback("frontal dominant axis not z")
    stepsL = np.abs(np.diff(voxL.astype(np.int64), axis=1)).mean((0, 1))
    if int(np.argmax(stepsL)) != 0:
        raise _GeometryFallback("lateral dominant axis not x")

    # frontal: slice axis c2; row table -> x (c0), col table -> y (c1)
    hitsF, rowF, colF, devF = _view_tables(voxF, okF, 2, 1, 0)
    # lateral: slice axis c0; row table -> y (c1), col table -> z (c2)
    hitsL, rowL, colL, devL = _view_tables(voxL, okL, 0, 2, 1)
    if devF + devL > 2000:
        raise _GeometryFallback(f"separability violated ({devF}+{devL})")

    HF, HL = len(hitsF), len(hitsL)
    if HF == 0 or HL == 0 or HF > V or HL > V:
        raise _GeometryFallback("degenerate hit sets")
    SLOTS = -(-max(HF, HL) // N_CORES)
    WR = N_CORES * SLOTS

    # global z-permutation: frontal hit slices rank-first
    permzF = np.full(V, -1, dtype=np.int64)
    for r, k in enumerate(hitsF):
        permzF[k] = r
    nxt = HF
    for k in range(V):
        if permzF[k] < 0:
            permzF[k] = nxt
            nxt += 1
    zinv = np.argsort(permzF)          # zp -> z

    # per-core inputs
    in_maps = []
    for c in range(N_CORES):
        rc_arr = np.full((128, 2, SLOTS, 2), POISON, dtype=f32)
        gtp_arr = np.zeros((128, 2, SLOTS, 128), dtype=f32)
        rx_arr = np.full((128, 2, WR), POISON, dtype=f32)
        # own-lateral-slot selector: x -> xp (= slot if x is mine else SLOTS)
        permxLc = np.full(V, SLOTS, dtype=np.int64)
        for s in range(SLOTS):
            xr = c + N_CORES * s
            if xr < HL:
                permxLc[hitsL[xr]] = s
        for s in range(SLOTS):
            r = c + N_CORES * s
            if r < HF:                  # own frontal slice rank r
                rc_arr[:, 0, s, 0] = rowF[:, r]            # x raw
                rc_arr[:, 0, s, 1] = colF[:, r]            # y = c1 raw
                z = hitsF[r]
                gtp_arr[:, 0, s, :] = gt[:, :, z].T + f32(QC)
            if r < HL:                  # own lateral slice rank r
                rc_arr[:, 1, s, 0] = rowL[:, r]            # y = c1 raw
                rc_arr[:, 1, s, 1] = _perm_table(colL[:, r],
                                                 permzF)   # z -> zp
                x = hitsL[r]
                gtp_arr[:, 1, s, :] = gt[x][:, zinv] + f32(QC)
        for r in range(min(HF, WR)):    # FX tables for ALL frontal ranks
            rx_arr[:, 0, r] = _perm_table(rowF[:, r], permxLc)
            rx_arr[:, 1, r] = colF[:, r]
        mk_arr = np.zeros((128, 4, 128), dtype=f32)
        for b in range(2):
            mk_arr[:, 2 * b + 0, :] = (pf[b, 0] > 0.5)
            mk_arr[:, 2 * b + 1, :] = (pl[b, 0] > 0.5)
        in_maps.append({
            "masks": mk_arr.astype(mybir.dt.np(BF16)),
            "rc": rc_arr,
            "rx": rx_arr,
            "gtp": gtp_arr,
        })
    return in_maps, (tuple(hitsF), tuple(hitsL))


def _combine(out_vecs, slots):
    """Host combine of per-core partial sums (float64)."""
    ov = np.asarray(out_vecs, dtype=np.float64)   # [cores, 128, NCOL]
    S1 = ov[:, :, 0:4 * slots].sum()              # gt'-dots (both views)
    S2 = ov[:, :, 4 * slots:6 * slots].sum()      # |F&L| partials
    N = float(V) ** 3
    total = 2.0 * N * Q0 + S1 + 2.0 * Q2 * S2
    return np.float32(-total / (2.0 * N))


def _reference_fallback(inputs):
    """Faithful f32 numpy replica of the jax reference (safety net)."""
    f32 = np.float32
    pf = np.asarray(inputs["pred_frontal"], dtype=f32)
    pl = np.asarray(inputs["pred_lateral"], dtype=f32)
    srcF = np.asarray(inputs["source_F"], dtype=f32)[0]
    tgtF = np.asarray(inputs["target_F"], dtype=f32)[0]
    srcL = np.asarray(inputs["source_L"], dtype=f32)[0]
    tgtL = np.asarray(inputs["target_L"], dtype=f32)[0]
    A_inv = np.asarray(inputs["A_inv"], dtype=f32)
    t_inv = np.asarray(inputs["t_inv"], dtype=f32)
    gt = np.asarray(inputs["vol_gt_3d"], dtype=f32)

    def backproject(mask2d, src, tgt):
        vox, ok = _trace_view(src, tgt, A_inv, t_inv)
        active = (mask2d > 0.5).reshape(-1)
        okm = ok & active[:, None]
        vi = np.clip(vox, 0, V - 1)
        vol = np.zeros((V, V, V), dtype=f32)
        flat = (vi[..., 0] * V + vi[..., 1]) * V + vi[..., 2]
        vol.reshape(-1)[flat[okm]] = 1.0
        return vol

    total = 0.0
    B = pf.shape[0]
    for b in range(B):
        vF = backproject(pf[b, 0], srcF, tgtF)
        vL = backproject(pl[b, 0], srcL, tgtL)
        sv = (vF + vL).astype(np.float64)
        p = 1.0 / (1.0 + np.exp(-sv))
        total += -(gt * np.log(p) + (1.0 - gt) * np.log1p(-p)).mean()
    return np.float32(total / B)


def kernel(**inputs) -> np.ndarray:
    try:
        in_maps, key = _host_prep(inputs)
    except _GeometryFallback:
        return _reference_fallback(inputs)
    nc = _build_program(key)
    res = run_bass_kernel_spmd(nc, in_maps, list(range(N_CORES)))
    slots = -(-max(len(key[0]), len(key[1])) // N_CORES)
    return _combine([r["out_vec"] for r in res.results], slots)
